# revision 1
# baseline (speedup 1.0000x reference)
"""Trainium2 Bass kernel for nn_Kernel3D (Gaussian splat onto a 64x64x64x8 grid).

Math:  out[x,y,z,t] = sum_n bx[n,x] * by[n,y] * bz[n,z] * x[n,t]
where b?[n,g] = exp(-0.5*((g-mu)/s)^2) / sqrt(2*pi*s^2)  (normalized Gaussian basis).

Strategy: shard the output X dimension across the 8 cores (8 x-planes each).
Per core the computation is one dense matmul
    out[(x y), (t z)] = P[n, (x y)]^T @ Q[n, (t z)]
with P[n, x*64+y] = bx[n,x]*by[n,y] (built as exp(-0.5*(ux^2+uy^2)) on chip)
and  Q[n, t*64+z] = (x[n,t]*Cn) * bz[n,z], Cn = (2*pi)^-1.5/(sx*sy*sz).
Contraction over n runs in chunks of 128 points (PSUM accumulation).
Each core only needs the points whose x-Gaussian overlaps its 8-voxel slab,
so points are binned per core host-side (pure sharding, no host math on values).
"""

import os
import sys

import numpy as np

for _p in ("/opt/trn_rl_repo", "/root/.axon_site/_ro/trn_rl_repo"):
    if os.path.isdir(_p) and _p not in sys.path:
        sys.path.insert(0, _p)

N_CORES = 8
GX, GY, GZ, GT = 64, 64, 64, 8
XPER = GX // N_CORES  # x-planes per core
PPC = 128  # points per chunk (partition dim)
FEAT = 16  # packed per-point features: x[8], mu[3], sigma[3], pad[2]

# Point selection: keep a point for a core if its x-Gaussian reaches the
# core's slab within SIGMA_CUT sigmas. exp(-0.5*4.5^2) ~ 4e-5 -> negligible.
SIGMA_CUT = 4.5
SELECT_POINTS = True

MM_DTYPE = "float32r"  # matmul input dtype: float32r = 1 cycle/row on trn2

_prog_cache = {}


def _build(n_chunks, mm_dt_name):
    import concourse.bass as bass
    import concourse.tile as tile
    from concourse import mybir
    from contextlib import ExitStack

    f32 = mybir.dt.float32
    mm_dt = getattr(mybir.dt, mm_dt_name)
    AL = mybir.AluOpType
    ACTF = mybir.ActivationFunctionType
    C0 = float((2.0 * np.pi) ** -1.5)

    nc = bass.Bass(use_seq_codegen=True)
    pts = nc.declare_dram_parameter("pts", [PPC, n_chunks * FEAT], f32, isOutput=False)
    xgrid = nc.declare_dram_parameter("xgrid", [PPC, XPER], f32, isOutput=False)
    iotayz = nc.declare_dram_parameter("iotayz", [PPC, GY], f32, isOutput=False)
    out = nc.declare_dram_parameter("out", [XPER * GY, GT * GZ], f32, isOutput=True)

    with tile.TileContext(nc) as tc, ExitStack() as ctx:
        cpool = ctx.enter_context(tc.tile_pool(name="const", bufs=1))
        wpool = ctx.enter_context(tc.tile_pool(name="work", bufs=3))
        opool = ctx.enter_context(tc.tile_pool(name="outp", bufs=2))
        ppool = ctx.enter_context(tc.tile_pool(name="accp", bufs=1, space="PSUM"))

        pts_t = cpool.tile([PPC, n_chunks * FEAT], f32, name="pts_t")
        nc.sync.dma_start(pts_t[:, :], pts[:, :])
        xg_t = cpool.tile([PPC, XPER], f32, name="xg_t")
        nc.sync.dma_start(xg_t[:, :], xgrid[:, :])
        io_t = cpool.tile([PPC, GY], f32, name="io_t")
        nc.sync.dma_start(io_t[:, :], iotayz[:, :])

        pts3 = pts_t[:, :].rearrange("p (c f) -> p c f", f=FEAT)

        # Batched per-point scalars for all chunks at once:
        #   inv_s = 1/sigma;  m2 = C0/(sx*sy*sz);  xc[n,t] = x[n,t]*m2[n]
        inv_t = cpool.tile([PPC, n_chunks, 3], f32, name="inv_t")
        nc.vector.reciprocal(inv_t[:, :, :], pts3[:, :, 11:14])
        m1_t = cpool.tile([PPC, n_chunks], f32, name="m1_t")
        nc.vector.tensor_tensor(m1_t[:, :], inv_t[:, :, 0], inv_t[:, :, 1], AL.mult)
        m2_t = cpool.tile([PPC, n_chunks], f32, name="m2_t")
        nc.vector.scalar_tensor_tensor(
            m2_t[:, :], m1_t[:, :], C0, inv_t[:, :, 2], AL.mult, AL.mult
        )
        xc_t = cpool.tile([PPC, n_chunks, GT], f32, name="xc_t")
        nc.vector.tensor_tensor(
            xc_t[:, :, :],
            pts3[:, :, 0:GT],
            m2_t[:, :].unsqueeze(2).broadcast_to((PPC, n_chunks, GT)),
            AL.mult,
        )

        accs = [
            ppool.tile([128, 512], f32, tag=f"acc{m}", name=f"acc{m}") for m in range(4)
        ]

        for c in range(n_chunks):
            mu_x = pts3[:, c, 8:9]
            mu_y = pts3[:, c, 9:10]
            mu_z = pts3[:, c, 10:11]
            ivx = inv_t[:, c, 0:1]
            ivy = inv_t[:, c, 1:2]
            ivz = inv_t[:, c, 2:3]

            # u = [(xg-mux)/sx | (yg-muy)/sy | (zg-muz)/sz], 136 wide, on DVE
            u_t = wpool.tile([PPC, 136], f32, name="u_t", tag="ubuf")
            nc.vector.scalar_tensor_tensor(
                u_t[:, 0:8], xg_t[:, :], mu_x, ivx.broadcast_to((PPC, XPER)),
                AL.subtract, AL.mult,
            )
            nc.vector.scalar_tensor_tensor(
                u_t[:, 8:72], io_t[:, :], mu_y, ivy.broadcast_to((PPC, GY)),
                AL.subtract, AL.mult,
            )
            nc.vector.scalar_tensor_tensor(
                u_t[:, 72:136], io_t[:, :], mu_z, ivz.broadcast_to((PPC, GZ)),
                AL.subtract, AL.mult,
            )
            # b = exp(-0.5*u^2): square then exp, both on ACT (single producer)
            sq_t = wpool.tile([PPC, 136], f32, name="sq_t", tag="sqbuf")
            nc.scalar.activation(sq_t[:, :], u_t[:, :], ACTF.Square)
            b_t = wpool.tile([PPC, 136], f32, name="b_t", tag="bbuf")
            nc.scalar.activation(b_t[:, :], sq_t[:, :], ACTF.Exp, scale=-0.5)

            # P[n, j*64+y] = bx[n,j]*by[n,y];  Q[n, t*64+z] = xc[n,t]*bz[n,z]
            # both built on DVE so the matmul has a single producer engine
            p_t = wpool.tile([PPC, 512], mm_dt, name="p_t", tag="pbuf")
            nc.vector.tensor_tensor(
                p_t[:, :].rearrange("p (a b) -> p a b", b=GY),
                b_t[:, 0:8].unsqueeze(2).broadcast_to((PPC, XPER, GY)),
                b_t[:, 8:72].unsqueeze(1).broadcast_to((PPC, XPER, GY)),
                AL.mult,
            )
            q_t = wpool.tile([PPC, 512], mm_dt, name="q_t", tag="qbuf")
            nc.vector.tensor_tensor(
                q_t[:, :].rearrange("p (a b) -> p a b", b=GZ),
                xc_t[:, c, :].unsqueeze(2).broadcast_to((PPC, GT, GZ)),
                b_t[:, 72:136].unsqueeze(1).broadcast_to((PPC, GT, GZ)),
                AL.mult,
            )

            for m in range(4):
                nc.tensor.matmul(
                    accs[m][:, :],
                    lhsT=p_t[:, m * 128 : (m + 1) * 128],
                    rhs=q_t[:, :],
                    start=(c == 0),
                    stop=(c == n_chunks - 1),
                )

        for m in range(4):
            o_t = opool.tile([128, 512], f32, name="o_t", tag="obuf")
            nc.scalar.copy(o_t[:, :], accs[m][:, :])
            nc.sync.dma_start(out[m * 128 : (m + 1) * 128, :], o_t[:, :])

    _split_multi_waits(nc, mybir)
    return nc


def _split_multi_waits(nc, mybir):
    """This walrus build rejects instructions carrying >1 sync-wait command.
    Hoist extra waits onto standalone same-engine InstEventSemaphore
    instructions inserted immediately before the overloaded instruction —
    identical semantics (sequencer blocks on each wait in program order)."""
    k = 0
    for bb in nc.m.functions[0].blocks:
        new = []
        for inst in bb.instructions:
            si = inst.sync_info
            if si is not None and si.on_wait and len(si.on_wait) > 1:
                for w in si.on_wait[:-1]:
                    wi = mybir.InstEventSemaphore(
                        name=f"wsplit_{k}", ins=[], outs=[]
                    )
                    k += 1
                    wi.engine = inst.engine
                    wi.sync_info = mybir.SyncInfo(on_wait=[w], on_update=[])
                    nc.register_instruction(wi)
                    new.append(wi)
                inst.sync_info = mybir.SyncInfo(
                    on_wait=[si.on_wait[-1]], on_update=si.on_update
                )
            new.append(inst)
        bb.instructions[:] = new


def _get_prog(n_chunks, mm_dt_name):
    key = (n_chunks, mm_dt_name)
    if key not in _prog_cache:
        _prog_cache[key] = _build(n_chunks, mm_dt_name)
    return _prog_cache[key]


def _pack_points(x, mu, sigma, n_chunks):
    """[n,8]+[n,3]+[n,3] -> [128, n_chunks*16] chunk-packed layout.

    Padding rows use sigma=1 / x=0 so they contribute exactly zero and
    produce no NaN/Inf anywhere in the pipeline.
    """
    n = x.shape[0]
    cap = n_chunks * PPC
    feat = np.zeros((cap, FEAT), np.float32)
    feat[:, 11:14] = 1.0  # sigma=1 for padding rows
    feat[:n, 0:8] = x
    feat[:n, 8:11] = mu
    feat[:n, 11:14] = sigma
    return (
        feat.reshape(n_chunks, PPC, FEAT).transpose(1, 0, 2).reshape(PPC, n_chunks * FEAT)
    )


def _prepare(x, mu, sigma):
    n = x.shape[0]
    if SELECT_POINTS:
        sel = []
        for c in range(N_CORES):
            lo, hi = c * XPER, c * XPER + XPER - 1  # inclusive grid range
            d = np.maximum.reduce([lo - mu[:, 0], mu[:, 0] - hi, np.zeros(n, np.float32)])
            sel.append(np.nonzero(d <= SIGMA_CUT * sigma[:, 0])[0])
        n_chunks = max(1, int(np.ceil(max(len(s) for s in sel) / PPC)))
    else:
        sel = [np.arange(n) for _ in range(N_CORES)]
        n_chunks = (n + PPC - 1) // PPC

    iota = np.tile(np.arange(GY, dtype=np.float32), (PPC, 1))
    in_maps = []
    for c in range(N_CORES):
        idx = sel[c]
        in_maps.append(
            {
                "pts": _pack_points(x[idx], mu[idx], sigma[idx], n_chunks),
                "xgrid": np.tile(
                    np.arange(c * XPER, (c + 1) * XPER, dtype=np.float32), (PPC, 1)
                ),
                "iotayz": iota,
            }
        )
    return in_maps, n_chunks


def _assemble(results):
    o = np.stack([results[c]["out"] for c in range(N_CORES)])  # [8, 512, 512]
    o = o.reshape(N_CORES, XPER, GY, GT, GZ).transpose(0, 1, 2, 4, 3)
    return np.ascontiguousarray(o.reshape(GX, GY, GZ, GT))


def run(x, mu, sigma, trace=False, **spmd_kwargs):
    """Returns (output, BassKernelResults)."""
    from concourse.bass_utils import run_bass_kernel_spmd

    x = np.asarray(x, np.float32)
    mu = np.asarray(mu, np.float32)
    sigma = np.asarray(sigma, np.float32)
    in_maps, n_chunks = _prepare(x, mu, sigma)
    nc = _get_prog(n_chunks, MM_DTYPE)
    res = run_bass_kernel_spmd(
        nc, in_maps, list(range(N_CORES)), trace=trace, **spmd_kwargs
    )
    return _assemble(res.results), res


def kernel(x, mu, sigma):
    out, _ = run(x, mu, sigma)
    return out



# revision 2
# speedup vs baseline: 1.1224x; 1.1224x over previous
"""Trainium2 Bass kernel for nn_Kernel3D (Gaussian splat onto a 64x64x64x8 grid).

Math:  out[x,y,z,t] = sum_n bx[n,x] * by[n,y] * bz[n,z] * x[n,t]
where b?[n,g] = exp(-0.5*((g-mu)/s)^2) / sqrt(2*pi*s^2)  (normalized Gaussian basis).

Strategy (v2): shard the output X dimension across the 8 cores (8 x-planes each).
Per core the computation is a PSUM-accumulated matmul over point-chunks of 128:
    out[(x y), (z t)] += P[n, (x y)]^T @ Q[n, (z t)]
with P[n, x*64+y] = bx[n,x]*by[n,y] and Q[n, z*8+t] = bz[n,z] * (x[n,t]*Cn),
Cn = (2*pi)^-1.5/(sx*sy*sz).

Points are binned per core (x-Gaussian overlaps the slab within SIGMA_CUT
sigmas) and sorted by mu_z.  Each chunk then only covers a narrow z-window
(sorted points + Gaussian reach), so Q and the matmul free dim shrink from
512 to ~wz*8 ~ 200 columns.  The z-window layout (z0/wz per chunk) is shared
across all 8 cores (union of per-core extents) because the SPMD program is
shared; chunk 0 uses the full window so PSUM has_written is initialized
everywhere.

Elementwise work is spread across engines:
  DVE    u=(g-mu)*iv via tensor_scalar (2 per-partition scalars, 2x mode),
         squares (fp16 TT, 2x), P outer-product TT
  ACT    batched exp over flat [x|y|zwin] segments; for some chunks a fused
         broadcast-exp produces bx replicated along y so the P TT runs at 2x
  GPSIMD Q outer-product (bz bcast-t times xc bcast-z)
  PE     4 weight blocks x (wz*8) free columns, fp16, PSUM z-offset writes
"""

import os
import sys

import numpy as np

for _p in ("/opt/trn_rl_repo", "/root/.axon_site/_ro/trn_rl_repo"):
    if os.path.isdir(_p) and _p not in sys.path:
        sys.path.insert(0, _p)

N_CORES = 8
GX, GY, GZ, GT = 64, 64, 64, 8
XPER = GX // N_CORES  # x-planes per core
PPC = 128  # points per chunk (partition dim)
FEAT = 16  # packed per-point features: x[8], mu[3], sigma[3], pad[2]

SIGMA_CUT = 3.0  # keep/window cut in sigmas; exp(-0.5*3^2) ~ 1.1e-2 edge value
ACT_BXR_FRAC = 0.5  # fraction of chunks whose bx-replication runs on ACT

_prog_cache = {}


def _build(n_chunks, z0s, wzs):
    import concourse.bass as bass
    import concourse.tile as tile
    from concourse import mybir
    from contextlib import ExitStack

    f32 = mybir.dt.float32
    f16 = mybir.dt.float16
    AL = mybir.AluOpType
    ACTF = mybir.ActivationFunctionType
    C0 = float((2.0 * np.pi) ** -1.5)

    # flat segment offsets: per chunk [x(8) | y(64) | zwin(wz)]
    segL = [8 + GY + wzs[c] for c in range(n_chunks)]
    off = np.concatenate([[0], np.cumsum(segL)]).astype(int)
    total_L = int(off[-1])

    # chunks whose bx-replication is produced by ACT (fused bcast-exp);
    # spread them evenly
    n_act = int(round(n_chunks * ACT_BXR_FRAC))
    act_bxr = set(np.linspace(0, n_chunks - 1, n_act).astype(int).tolist()) if n_act else set()

    nc = bass.Bass(use_seq_codegen=True)
    pts = nc.declare_dram_parameter("pts", [PPC, n_chunks * FEAT], f32, isOutput=False)
    gridcat = nc.declare_dram_parameter("gridcat", [PPC, total_L], f32, isOutput=False)
    out = nc.declare_dram_parameter("out", [XPER * GY, GZ * GT], f32, isOutput=True)

    with tile.TileContext(nc) as tc, ExitStack() as ctx:
        cpool = ctx.enter_context(tc.tile_pool(name="const", bufs=1))
        wpool = ctx.enter_context(tc.tile_pool(name="work", bufs=3))
        opool = ctx.enter_context(tc.tile_pool(name="outp", bufs=2))
        ppool = ctx.enter_context(tc.tile_pool(name="accp", bufs=1, space="PSUM"))

        pts_t = cpool.tile([PPC, n_chunks * FEAT], f32, name="pts_t")
        nc.sync.dma_start(pts_t[:, :], pts[:, :])
        grid_t = cpool.tile([PPC, total_L], f32, name="grid_t")
        nc.sync.dma_start(grid_t[:, :], gridcat[:, :])

        pts3 = pts_t[:, :].rearrange("p (c f) -> p c f", f=FEAT)

        # per-point scalars, batched over all chunks
        inv_t = cpool.tile([PPC, n_chunks, 3], f32, name="inv_t")
        nc.vector.reciprocal(inv_t[:, :, :], pts3[:, :, 11:14])
        ivzc_t = cpool.tile([PPC, n_chunks], f32, name="ivzc_t")
        nc.vector.tensor_scalar(ivzc_t[:, :], inv_t[:, :, 2], C0, None, AL.mult)
        m1_t = cpool.tile([PPC, n_chunks], f32, name="m1_t")
        nc.vector.tensor_tensor(m1_t[:, :], inv_t[:, :, 0], inv_t[:, :, 1], AL.mult)
        m2_t = cpool.tile([PPC, n_chunks], f32, name="m2_t")
        nc.vector.tensor_tensor(m2_t[:, :], m1_t[:, :], ivzc_t[:, :], AL.mult)
        # xc[p,c,t] = x * C0/(sx sy sz), fp16
        xc_t = cpool.tile([PPC, n_chunks, GT], f16, name="xc_t")
        nc.vector.tensor_tensor(
            xc_t[:, :, :],
            pts3[:, :, 0:GT],
            m2_t[:, :].unsqueeze(2).broadcast_to((PPC, n_chunks, GT)),
            AL.mult,
        )

        # flat per-chunk [x|y|zw] working tiles
        u_t = cpool.tile([PPC, total_L], f16, name="u_t")
        usq_t = cpool.tile([PPC, total_L], f16, name="usq_t")
        b_t = cpool.tile([PPC, total_L], f16, name="b_t")

        accs = [
            ppool.tile([128, 512], f32, tag=f"acc{m}", name=f"acc{m}") for m in range(4)
        ]

        for c in range(n_chunks):
            o = int(off[c])
            wz = int(wzs[c])
            L = segL[c]
            mu_x = pts3[:, c, 8:9]
            mu_y = pts3[:, c, 9:10]
            mu_z = pts3[:, c, 10:11]
            ivx = inv_t[:, c, 0:1]
            ivy = inv_t[:, c, 1:2]
            ivz = inv_t[:, c, 2:3]

            # u = (g - mu) * iv  per dim (tensor_scalar: 2 per-partition scalars)
            nc.vector.tensor_scalar(
                u_t[:, o : o + 8], grid_t[:, o : o + 8], mu_x, ivx, AL.subtract, AL.mult
            )
            nc.vector.tensor_scalar(
                u_t[:, o + 8 : o + 8 + GY], grid_t[:, o + 8 : o + 8 + GY],
                mu_y, ivy, AL.subtract, AL.mult,
            )
            nc.vector.tensor_scalar(
                u_t[:, o + 8 + GY : o + L], grid_t[:, o + 8 + GY : o + L],
                mu_z, ivz, AL.subtract, AL.mult,
            )
            # usq = u*u (fp16 2x)
            nc.vector.tensor_tensor(
                usq_t[:, o : o + L], u_t[:, o : o + L], u_t[:, o : o + L], AL.mult
            )
            # b = exp(-0.5 usq)
            nc.scalar.activation(
                b_t[:, o : o + L], usq_t[:, o : o + L], ACTF.Exp, scale=-0.5
            )

            # P[p, x, y] = bx * by
            p_t = wpool.tile([PPC, XPER, GY], f16, name="p_t", tag="pbuf")
            if c in act_bxr:
                # ACT produces bx replicated along y (fused bcast exp)
                bxr_t = wpool.tile([PPC, XPER, GY], f16, name="bxr_t", tag="bxrbuf")
                nc.scalar.activation(
                    bxr_t[:, :, :],
                    usq_t[:, o : o + 8].unsqueeze(2).broadcast_to((PPC, XPER, GY)),
                    ACTF.Exp,
                    scale=-0.5,
                )
                nc.vector.tensor_tensor(
                    p_t[:, :, :],
                    bxr_t[:, :, :],
                    b_t[:, o + 8 : o + 8 + GY].unsqueeze(1).broadcast_to(
                        (PPC, XPER, GY)
                    ),
                    AL.mult,
                )
            else:
                nc.vector.tensor_tensor(
                    p_t[:, :, :],
                    b_t[:, o : o + 8].unsqueeze(2).broadcast_to((PPC, XPER, GY)),
                    b_t[:, o + 8 : o + 8 + GY].unsqueeze(1).broadcast_to(
                        (PPC, XPER, GY)
                    ),
                    AL.mult,
                )

            # Q[p, z, t] = bz * xc   (on GPSIMD; both broadcasts are legal APs)
            q_t = wpool.tile([PPC, wz, GT], f16, name="q_t", tag="qbuf")
            nc.gpsimd.tensor_tensor(
                q_t[:, :, :],
                b_t[:, o + 8 + GY : o + L].unsqueeze(2).broadcast_to((PPC, wz, GT)),
                xc_t[:, c, :].unsqueeze(1).broadcast_to((PPC, wz, GT)),
                AL.mult,
            )

            z0 = int(z0s[c])
            pf = p_t[:, :, :].rearrange("p a b -> p (a b)")
            qf = q_t[:, :, :].rearrange("p a b -> p (a b)")
            for m in range(4):
                nc.tensor.matmul(
                    accs[m][:, z0 * GT : (z0 + wz) * GT],
                    lhsT=pf[:, m * 128 : (m + 1) * 128],
                    rhs=qf[:, :],
                    start=(c == 0),
                    stop=(c == n_chunks - 1),
                )

        for m in range(4):
            o_t = opool.tile([128, 512], f32, name="o_t", tag="obuf")
            nc.scalar.copy(o_t[:, :], accs[m][:, :])
            nc.sync.dma_start(out[m * 128 : (m + 1) * 128, :], o_t[:, :])

    _split_multi_waits(nc, mybir)
    return nc


def _split_multi_waits(nc, mybir):
    """This walrus build rejects instructions carrying >1 sync-wait command.
    Hoist extra waits onto standalone same-engine InstEventSemaphore
    instructions inserted immediately before the overloaded instruction."""
    k = 0
    for bb in nc.m.functions[0].blocks:
        new = []
        for inst in bb.instructions:
            si = inst.sync_info
            if si is not None and si.on_wait and len(si.on_wait) > 1:
                for w in si.on_wait[:-1]:
                    wi = mybir.InstEventSemaphore(name=f"wsplit_{k}", ins=[], outs=[])
                    k += 1
                    wi.engine = inst.engine
                    wi.sync_info = mybir.SyncInfo(on_wait=[w], on_update=[])
                    nc.register_instruction(wi)
                    new.append(wi)
                inst.sync_info = mybir.SyncInfo(
                    on_wait=[si.on_wait[-1]], on_update=si.on_update
                )
            new.append(inst)
        bb.instructions[:] = new


def _get_prog(n_chunks, z0s, wzs):
    key = (n_chunks, tuple(z0s), tuple(wzs))
    if key not in _prog_cache:
        _prog_cache[key] = _build(n_chunks, z0s, wzs)
    return _prog_cache[key]


def _pack_points(x, mu, sigma, n_chunks, z0s, wzs, core):
    """[n,8]+[n,3]+[n,3] -> [128, n_chunks*16] chunk-packed layout.
    Padding rows use sigma=1 / x=0, mu centered in the chunk window."""
    n = x.shape[0]
    cap = n_chunks * PPC
    feat = np.zeros((cap, FEAT), np.float32)
    feat[:, 11:14] = 1.0
    for c in range(n_chunks):
        feat[c * PPC : (c + 1) * PPC, 8] = core * XPER + XPER / 2.0
        feat[c * PPC : (c + 1) * PPC, 9] = GY / 2.0
        feat[c * PPC : (c + 1) * PPC, 10] = z0s[c] + wzs[c] / 2.0
    feat[:n, 0:8] = x
    feat[:n, 8:11] = mu
    feat[:n, 11:14] = sigma
    return (
        feat.reshape(n_chunks, PPC, FEAT).transpose(1, 0, 2).reshape(PPC, n_chunks * FEAT)
    )


def _prepare(x, mu, sigma):
    n = x.shape[0]
    C = SIGMA_CUT
    sel = []
    for c in range(N_CORES):
        lo, hi = c * XPER, c * XPER + XPER - 1
        d = np.maximum.reduce([lo - mu[:, 0], mu[:, 0] - hi, np.zeros(n, np.float32)])
        idx = np.nonzero(d <= C * sigma[:, 0])[0]
        idx = idx[np.argsort(mu[idx, 2], kind="stable")]  # sort by mu_z
        sel.append(idx)
    n_chunks = max(1, int(np.ceil(max(len(s) for s in sel) / PPC)))

    # shared-across-cores per-chunk z windows (union of per-core extents)
    z0s, z1s = [], []
    for c in range(n_chunks):
        zlo, zhi = GZ, 0
        for k in range(N_CORES):
            idx = sel[k][c * PPC : (c + 1) * PPC]
            if len(idx) == 0:
                continue
            zlo = min(zlo, np.min(mu[idx, 2] - C * sigma[idx, 2]))
            zhi = max(zhi, np.max(mu[idx, 2] + C * sigma[idx, 2]))
        z0 = max(0, int(np.floor(zlo)))
        z1 = min(GZ, int(np.ceil(zhi)))
        if z1 <= z0:
            z0, z1 = 0, 1
        z0s.append(z0)
        z1s.append(z1)
    z0s[0], z1s[0] = 0, GZ  # chunk 0 full width: initializes PSUM has_written
    wzs = [z1s[c] - z0s[c] for c in range(n_chunks)]

    # per-chunk grid constant [x(8) | y(64) | z-window]
    segs = []
    for c in range(n_chunks):
        segs.append(np.zeros(8, np.float32))  # per-core x filled below
        segs.append(np.arange(GY, dtype=np.float32))
        segs.append(np.arange(z0s[c], z1s[c], dtype=np.float32))
    in_maps = []
    for k in range(N_CORES):
        g = []
        for c in range(n_chunks):
            g.append(np.arange(k * XPER, (k + 1) * XPER, dtype=np.float32))
            g.append(np.arange(GY, dtype=np.float32))
            g.append(np.arange(z0s[c], z1s[c], dtype=np.float32))
        gridcat = np.tile(np.concatenate(g), (PPC, 1))
        idx = sel[k]
        in_maps.append(
            {
                "pts": _pack_points(
                    x[idx], mu[idx], sigma[idx], n_chunks, z0s, wzs, k
                ),
                "gridcat": gridcat,
            }
        )
    return in_maps, n_chunks, z0s, wzs


def _assemble(results):
    o = np.stack([results[c]["out"] for c in range(N_CORES)])  # [8, 512, 512]
    # rows: x*64+y (x within slab), cols: z*8+t
    o = o.reshape(N_CORES, XPER, GY, GZ, GT)
    return np.ascontiguousarray(o.reshape(GX, GY, GZ, GT))


def run(x, mu, sigma, trace=False, **spmd_kwargs):
    """Returns (output, BassKernelResults)."""
    from concourse.bass_utils import run_bass_kernel_spmd

    x = np.asarray(x, np.float32)
    mu = np.asarray(mu, np.float32)
    sigma = np.asarray(sigma, np.float32)
    in_maps, n_chunks, z0s, wzs = _prepare(x, mu, sigma)
    nc = _get_prog(n_chunks, z0s, wzs)
    res = run_bass_kernel_spmd(
        nc, in_maps, list(range(N_CORES)), trace=trace, **spmd_kwargs
    )
    return _assemble(res.results), res


def kernel(x, mu, sigma):
    out, _ = run(x, mu, sigma)
    return out


# revision 4
# speedup vs baseline: 1.2820x; 1.1422x over previous
"""Trainium2 Bass kernel for nn_Kernel3D (Gaussian splat onto a 64x64x64x8 grid).

Math:  out[x,y,z,t] = sum_n bx[n,x] * by[n,y] * bz[n,z] * x[n,t]
where b?[n,g] = exp(-0.5*((g-mu)/s)^2) / sqrt(2*pi*s^2).

v3: x-slab sharding (8 planes/core) + z-sorted point chunks of 128 with a
uniform z-window width WZ (per-chunk start, shared across cores so the SPMD
program is identical).  Work is batched into large per-GROUP instructions
(TRN2 pays ~2.3x overhead on small SBUF ops and Tile semaphores scale with
instruction count) and groups are software-pipelined across engines:

  DVE   d = g - mu, sqd = d*d (fp16 2x), usq = sqd*iv^2, P/Q outer products
  ACT   batched exp; fused broadcast-exp produces the bx factor replicated
        along y for tail chunks (so their P TT runs at 2x); PSUM evacuation
  PE    4 zero-matmuls init PSUM; per chunk 4 blocks x (wz*8) fp16 columns
        accumulated at the chunk's z-offset
  GPSIMD memset only (measured ~4ns/elem on real ops - not worth using)

Host side: selection (slab dist <= C*sigma_x), z-sort, packing, window
metadata.  No host math on values.
"""

import os
import sys

import numpy as np

for _p in ("/opt/trn_rl_repo", "/root/.axon_site/_ro/trn_rl_repo"):
    if os.path.isdir(_p) and _p not in sys.path:
        sys.path.insert(0, _p)

N_CORES = 8
GX, GY, GZ, GT = 64, 64, 64, 8
XPER = GX // N_CORES
PPC = 128
FEAT = 16  # x[8], mu[3], sigma[3], pad[2]

SIGMA_CUT = 3.0
N_GROUPS = 3  # pipeline groups
ACT_BXR_FRAC = 1.0  # fraction of each group's chunks with ACT-built bx-replica
N_ACT_EVAC = 2  # PSUM quarters evacuated by ACT (rest DVE)

_prog_cache = {}


def _build(n_chunks, z0s, wzs, s0s, WZ):
    import concourse.bass as bass
    import concourse.tile as tile
    from concourse import mybir
    from contextlib import ExitStack

    f32 = mybir.dt.float32
    f16 = mybir.dt.float16
    AL = mybir.AluOpType
    ACTF = mybir.ActivationFunctionType
    C0 = float((2.0 * np.pi) ** -1.5)
    NC = n_chunks
    L = XPER + GY + WZ  # flat segment length per chunk: [x | y | zwin]

    # pipeline groups: contiguous chunk ranges
    G = min(N_GROUPS, NC)
    bounds = [round(g * NC / G) for g in range(G + 1)]
    groups = [(bounds[g], bounds[g + 1]) for g in range(G)]
    # per-group ACT-bxr tail counts and compact bxr slot ranges
    k_acts = [int(round((c1 - c0) * ACT_BXR_FRAC)) for c0, c1 in groups]
    slot0 = np.concatenate([[0], np.cumsum(k_acts)]).astype(int)
    K_TOT = int(slot0[-1])

    nc = bass.Bass(use_seq_codegen=True)
    pts = nc.declare_dram_parameter("pts", [PPC, NC * FEAT], f32, isOutput=False)
    gx_d = nc.declare_dram_parameter("gx", [PPC, XPER], f32, isOutput=False)
    gy_d = nc.declare_dram_parameter("gy", [PPC, GY], f32, isOutput=False)
    gz_d = nc.declare_dram_parameter("gz", [PPC, NC * WZ], f32, isOutput=False)
    out = nc.declare_dram_parameter("out", [XPER * GY, GZ * GT], f32, isOutput=True)

    with tile.TileContext(nc) as tc, ExitStack() as ctx:
        cpool = ctx.enter_context(tc.tile_pool(name="const", bufs=1))
        ppool = ctx.enter_context(tc.tile_pool(name="accp", bufs=1, space="PSUM"))

        # zero tile + ACT table prefetch (dummy exp) run during input DMA
        zero_t = cpool.tile([PPC, 640], f16, name="zero_t")
        nc.gpsimd.memset(zero_t[:, :], 0.0)
        dummy_t = cpool.tile([PPC, 1], f16, name="dummy_t")
        nc.scalar.activation(dummy_t[:, :], zero_t[:, 0:1], ACTF.Exp, scale=-0.5)

        pts_t = cpool.tile([PPC, NC * FEAT], f32, name="pts_t")
        nc.sync.dma_start(pts_t[:, :], pts[:, :])
        gx_t = cpool.tile([PPC, XPER], f32, name="gx_t")
        nc.sync.dma_start(gx_t[:, :], gx_d[:, :])
        gy_t = cpool.tile([PPC, GY], f32, name="gy_t")
        nc.sync.dma_start(gy_t[:, :], gy_d[:, :])
        gz_t = cpool.tile([PPC, NC * WZ], f32, name="gz_t")
        nc.sync.dma_start(gz_t[:, :], gz_d[:, :])

        pts3 = pts_t[:, :].rearrange("p (c f) -> p c f", f=FEAT)
        gz3 = gz_t[:, :].rearrange("p (c w) -> p c w", w=WZ)

        # PSUM: one contiguous [128, 2048] region = 4 banks; zero-matmul init
        acc = ppool.tile([128, 4 * 512], f32, name="acc")
        for m in range(4):
            nc.tensor.matmul(
                acc[:, m * 512 : (m + 1) * 512],
                lhsT=zero_t[:, 0:128],
                rhs=zero_t[:, 128:640],
                start=True,
                stop=False,
            )

        # per-point scalars (small batched ops)
        inv_t = cpool.tile([PPC, NC, 3], f32, name="inv_t")
        nc.vector.reciprocal(inv_t[:, :, :], pts3[:, :, 11:14])
        iv2_t = cpool.tile([PPC, NC, 3], f32, name="iv2_t")
        nc.vector.tensor_tensor(iv2_t[:, :, :], inv_t[:, :, :], inv_t[:, :, :], AL.mult)
        ivzc_t = cpool.tile([PPC, NC], f32, name="ivzc_t")
        nc.vector.tensor_scalar(ivzc_t[:, :], inv_t[:, :, 2], C0, None, AL.mult)
        m1_t = cpool.tile([PPC, NC], f32, name="m1_t")
        nc.vector.tensor_tensor(m1_t[:, :], inv_t[:, :, 0], inv_t[:, :, 1], AL.mult)
        m2_t = cpool.tile([PPC, NC], f32, name="m2_t")
        nc.vector.tensor_tensor(m2_t[:, :], m1_t[:, :], ivzc_t[:, :], AL.mult)
        xc_t = cpool.tile([PPC, NC, GT], f16, name="xc_t")
        nc.vector.tensor_tensor(
            xc_t[:, :, :],
            pts3[:, :, 0:GT],
            m2_t[:, :].unsqueeze(2).broadcast_to((PPC, NC, GT)),
            AL.mult,
        )

        d_t = cpool.tile([PPC, NC, L], f16, name="d_t")
        sq_t = cpool.tile([PPC, NC, L], f16, name="sq_t")
        usq_t = cpool.tile([PPC, NC, L], f16, name="usq_t")
        b_xy = cpool.tile([PPC, NC, XPER + GY], f16, name="b_xy")
        bzr_t = cpool.tile([PPC, NC, WZ, GT], f16, name="bzr_t")
        if K_TOT:
            bxr_t = cpool.tile([PPC, K_TOT, XPER, GY], f16, name="bxr_t")
        p_t = cpool.tile([PPC, NC, XPER, GY], f16, name="p_t")
        q_t = cpool.tile([PPC, NC, WZ, GT], f16, name="q_t")
        pf = p_t[:, :, :, :].rearrange("p c a b -> p c (a b)")
        qf = q_t[:, :, :, :].rearrange("p c a b -> p c (a b)")

        def emit_front(g):
            c0, c1 = groups[g]
            n = c1 - c0
            # d = g - mu (fp16 out)
            nc.vector.tensor_tensor(
                d_t[:, c0:c1, 0:XPER],
                gx_t[:, :].unsqueeze(1).broadcast_to((PPC, n, XPER)),
                pts3[:, c0:c1, 8:9].broadcast_to((PPC, n, XPER)),
                AL.subtract,
            )
            nc.vector.tensor_tensor(
                d_t[:, c0:c1, XPER : XPER + GY],
                gy_t[:, :].unsqueeze(1).broadcast_to((PPC, n, GY)),
                pts3[:, c0:c1, 9:10].broadcast_to((PPC, n, GY)),
                AL.subtract,
            )
            nc.vector.tensor_tensor(
                d_t[:, c0:c1, XPER + GY : L],
                gz3[:, c0:c1, :],
                pts3[:, c0:c1, 10:11].broadcast_to((PPC, n, WZ)),
                AL.subtract,
            )
            # sqd = d*d (2x), usq = sqd * iv2 (per-dim bcast)
            nc.vector.tensor_tensor(
                sq_t[:, c0:c1, :], d_t[:, c0:c1, :], d_t[:, c0:c1, :], AL.mult
            )
            nc.vector.tensor_tensor(
                usq_t[:, c0:c1, 0:XPER],
                sq_t[:, c0:c1, 0:XPER],
                iv2_t[:, c0:c1, 0:1].broadcast_to((PPC, n, XPER)),
                AL.mult,
            )
            nc.vector.tensor_tensor(
                usq_t[:, c0:c1, XPER : XPER + GY],
                sq_t[:, c0:c1, XPER : XPER + GY],
                iv2_t[:, c0:c1, 1:2].broadcast_to((PPC, n, GY)),
                AL.mult,
            )
            nc.vector.tensor_tensor(
                usq_t[:, c0:c1, XPER + GY : L],
                sq_t[:, c0:c1, XPER + GY : L],
                iv2_t[:, c0:c1, 2:3].broadcast_to((PPC, n, WZ)),
                AL.mult,
            )
            # ACT: compact exps + fused replicating exps
            nc.scalar.activation(
                b_xy[:, c0:c1, :], usq_t[:, c0:c1, 0 : XPER + GY], ACTF.Exp, scale=-0.5
            )
            nc.scalar.activation(
                bzr_t[:, c0:c1, :, :],
                usq_t[:, c0:c1, XPER + GY : L]
                .unsqueeze(3)
                .broadcast_to((PPC, n, WZ, GT)),
                ACTF.Exp,
                scale=-0.5,
            )
            k = k_acts[g]
            if k:
                nc.scalar.activation(
                    bxr_t[:, slot0[g] : slot0[g + 1], :, :],
                    usq_t[:, c1 - k : c1, 0:XPER]
                    .unsqueeze(3)
                    .broadcast_to((PPC, k, XPER, GY)),
                    ACTF.Exp,
                    scale=-0.5,
                )

        def emit_back(g):
            c0, c1 = groups[g]
            k = k_acts[g]
            kd = (c1 - c0) - k  # head chunks: P via 1x TT
            if kd:
                nc.vector.tensor_tensor(
                    p_t[:, c0 : c0 + kd, :, :],
                    b_xy[:, c0 : c0 + kd, 0:XPER]
                    .unsqueeze(3)
                    .broadcast_to((PPC, kd, XPER, GY)),
                    b_xy[:, c0 : c0 + kd, XPER : XPER + GY]
                    .unsqueeze(2)
                    .broadcast_to((PPC, kd, XPER, GY)),
                    AL.mult,
                )
            if k:
                nc.vector.tensor_tensor(
                    p_t[:, c1 - k : c1, :, :],
                    bxr_t[:, slot0[g] : slot0[g + 1], :, :],
                    b_xy[:, c1 - k : c1, XPER : XPER + GY]
                    .unsqueeze(2)
                    .broadcast_to((PPC, k, XPER, GY)),
                    AL.mult,
                )
            n = c1 - c0
            nc.vector.tensor_tensor(
                q_t[:, c0:c1, :, :],
                bzr_t[:, c0:c1, :, :],
                xc_t[:, c0:c1, :].unsqueeze(2).broadcast_to((PPC, n, WZ, GT)),
                AL.mult,
            )
            for c in range(c0, c1):
                z0, wz, s0 = int(z0s[c]), int(wzs[c]), int(s0s[c])
                for m in range(4):
                    nc.tensor.matmul(
                        acc[:, m * 512 + z0 * GT : m * 512 + (z0 + wz) * GT],
                        lhsT=pf[:, c, m * 128 : (m + 1) * 128],
                        rhs=qf[:, c, s0 * GT : (s0 + wz) * GT],
                        start=False,
                        stop=(c == NC - 1),
                    )

        # software pipeline: front(g) ... back(g-1)
        emit_front(0)
        for g in range(1, G):
            emit_front(g)
            emit_back(g - 1)
        emit_back(G - 1)

        # evacuate + store
        o_t = cpool.tile([128, 4 * 512], f32, name="o_t")
        ne = min(N_ACT_EVAC, 4)
        if ne:
            nc.scalar.copy(o_t[:, 0 : ne * 512], acc[:, 0 : ne * 512])
        if ne < 4:
            nc.vector.tensor_copy(o_t[:, ne * 512 :], acc[:, ne * 512 :])
        for m in range(4):
            nc.sync.dma_start(
                out[m * 128 : (m + 1) * 128, :], o_t[:, m * 512 : (m + 1) * 512]
            )

    _split_multi_waits(nc, mybir)
    return nc


def _split_multi_waits(nc, mybir):
    """Walrus rejects instructions with >1 sync-wait; hoist extras onto
    standalone same-engine InstEventSemaphore instructions."""
    k = 0
    for bb in nc.m.functions[0].blocks:
        new = []
        for inst in bb.instructions:
            si = inst.sync_info
            if si is not None and si.on_wait and len(si.on_wait) > 1:
                for w in si.on_wait[:-1]:
                    wi = mybir.InstEventSemaphore(name=f"wsplit_{k}", ins=[], outs=[])
                    k += 1
                    wi.engine = inst.engine
                    wi.sync_info = mybir.SyncInfo(on_wait=[w], on_update=[])
                    nc.register_instruction(wi)
                    new.append(wi)
                inst.sync_info = mybir.SyncInfo(
                    on_wait=[si.on_wait[-1]], on_update=si.on_update
                )
            new.append(inst)
        bb.instructions[:] = new


def _get_prog(n_chunks, z0s, wzs, s0s, WZ):
    key = (
        n_chunks, tuple(z0s), tuple(wzs), tuple(s0s), WZ,
        N_GROUPS, ACT_BXR_FRAC, N_ACT_EVAC,
    )
    if key not in _prog_cache:
        _prog_cache[key] = _build(n_chunks, z0s, wzs, s0s, WZ)
    return _prog_cache[key]


def _pack_points(x, mu, sigma, n_chunks, z0s, wzs, core):
    n = x.shape[0]
    cap = n_chunks * PPC
    feat = np.zeros((cap, FEAT), np.float32)
    feat[:, 11:14] = 1.0
    for c in range(n_chunks):
        feat[c * PPC : (c + 1) * PPC, 8] = core * XPER + XPER / 2.0
        feat[c * PPC : (c + 1) * PPC, 9] = GY / 2.0
        feat[c * PPC : (c + 1) * PPC, 10] = z0s[c] + wzs[c] / 2.0
    feat[:n, 0:8] = x
    feat[:n, 8:11] = mu
    feat[:n, 11:14] = sigma
    return (
        feat.reshape(n_chunks, PPC, FEAT).transpose(1, 0, 2).reshape(PPC, n_chunks * FEAT)
    )


def _prepare(x, mu, sigma):
    n = x.shape[0]
    C = SIGMA_CUT
    sel = []
    for c in range(N_CORES):
        lo, hi = c * XPER, c * XPER + XPER - 1
        d = np.maximum.reduce([lo - mu[:, 0], mu[:, 0] - hi, np.zeros(n, np.float32)])
        idx = np.nonzero(d <= C * sigma[:, 0])[0]
        idx = idx[np.argsort(mu[idx, 2], kind="stable")]
        sel.append(idx)
    n_chunks = max(1, int(np.ceil(max(len(s) for s in sel) / PPC)))

    z0s, z1s = [], []
    for c in range(n_chunks):
        zlo, zhi = GZ, 0
        for k in range(N_CORES):
            idx = sel[k][c * PPC : (c + 1) * PPC]
            if len(idx):
                zlo = min(zlo, np.min(mu[idx, 2] - C * sigma[idx, 2]))
                zhi = max(zhi, np.max(mu[idx, 2] + C * sigma[idx, 2]))
        z0 = max(0, int(np.floor(zlo)))
        z1 = min(GZ, int(np.ceil(zhi)))
        if z1 <= z0:
            z0, z1 = 0, 1
        z0s.append(z0)
        z1s.append(z1)
    wzs = [z1s[c] - z0s[c] for c in range(n_chunks)]
    WZ = max(wzs)
    # window [g0, g0+WZ) starts at z0 unless clipped by the grid edge
    g0s = [z0s[c] if z0s[c] + WZ <= GZ else GZ - WZ for c in range(n_chunks)]
    s0s = [z0s[c] - g0s[c] for c in range(n_chunks)]

    gy = np.arange(GY, dtype=np.float32)
    in_maps = []
    for k in range(N_CORES):
        gz = np.concatenate(
            [np.arange(g0s[c], g0s[c] + WZ, dtype=np.float32) for c in range(n_chunks)]
        )
        idx = sel[k]
        in_maps.append(
            {
                "pts": _pack_points(x[idx], mu[idx], sigma[idx], n_chunks, z0s, wzs, k),
                "gx": np.tile(
                    np.arange(k * XPER, (k + 1) * XPER, dtype=np.float32), (PPC, 1)
                ),
                "gy": np.tile(gy, (PPC, 1)),
                "gz": np.tile(gz, (PPC, 1)),
            }
        )
    return in_maps, n_chunks, z0s, wzs, s0s, WZ


def _assemble(results):
    o = np.stack([results[c]["out"] for c in range(N_CORES)])  # [8, 512, 512]
    o = o.reshape(N_CORES, XPER, GY, GZ, GT)
    return np.ascontiguousarray(o.reshape(GX, GY, GZ, GT))


def run(x, mu, sigma, trace=False, **spmd_kwargs):
    from concourse.bass_utils import run_bass_kernel_spmd

    x = np.asarray(x, np.float32)
    mu = np.asarray(mu, np.float32)
    sigma = np.asarray(sigma, np.float32)
    in_maps, n_chunks, z0s, wzs, s0s, WZ = _prepare(x, mu, sigma)
    nc = _get_prog(n_chunks, z0s, wzs, s0s, WZ)
    res = run_bass_kernel_spmd(
        nc, in_maps, list(range(N_CORES)), trace=trace, **spmd_kwargs
    )
    return _assemble(res.results), res


def kernel(x, mu, sigma):
    out, _ = run(x, mu, sigma)
    return out


# revision 5
# speedup vs baseline: 1.3184x; 1.0284x over previous
"""Trainium2 Bass kernel for nn_Kernel3D (Gaussian splat onto a 64x64x64x8 grid).

Math:  out[x,y,z,t] = sum_n bx[n,x] * by[n,y] * bz[n,z] * x[n,t]
where b?[n,g] = exp(-0.5*((g-mu)/s)^2) / sqrt(2*pi*s^2).

v4: x-slab sharding (8 planes/core) + z-sorted point chunks of 128 with a
uniform z-window width WZ (chunk z-offsets shared across cores so the SPMD
program is identical).  Per chunk the accumulated matmul is
    out[(x y), (z t)] += P[n, (x y)]^T @ Q[n, (z0..z0+wz) t]
Work is batched into large per-GROUP instructions and pipelined:

  DVE   d = g - mu; sqd = d*d (fp16 2x); usq = sqd * iv2-replica (2x);
        bz-replica over t via an int32-bitcast pair/quad copy (2x_2p);
        P and Q outer-product TTs at 2x
  ACT   one batched exp per group; fused broadcast-exp building the bx
        factor replicated along y (so P's TT runs at 2x); PSUM evacuation
  PE    4 zero-matmuls init PSUM; per chunk 4 blocks x (wz*8) fp16 columns
  GPSIMD memset + the single input DMA (cheap queue issue)

Host side: selection (slab dist <= C*sigma_x), z-sort, packing, window
metadata.  No host math on values.
"""

import os
import sys

import numpy as np

for _p in ("/opt/trn_rl_repo", "/root/.axon_site/_ro/trn_rl_repo"):
    if os.path.isdir(_p) and _p not in sys.path:
        sys.path.insert(0, _p)

N_CORES = 8
GX, GY, GZ, GT = 64, 64, 64, 8
XPER = GX // N_CORES
PPC = 128
FEAT = 16  # x[8], mu[3], sigma[3], pad[2]

SIGMA_CUT = 3.0
N_GROUPS = 3

_prog_cache = {}


def _build(n_chunks, z0s, wzs, s0s, WZ):
    import concourse.bass as bass
    import concourse.tile as tile
    from concourse import mybir
    from contextlib import ExitStack

    f32 = mybir.dt.float32
    f16 = mybir.dt.float16
    u32 = mybir.dt.uint32
    AL = mybir.AluOpType
    ACTF = mybir.ActivationFunctionType
    C0 = float((2.0 * np.pi) ** -1.5)
    NC = n_chunks
    L = XPER + GY + WZ  # per-chunk flat segments [x | y | zwin]
    ZO = XPER + GY  # z segment offset

    G = min(N_GROUPS, NC)
    bounds = [round(g * NC / G) for g in range(G + 1)]
    groups = [(bounds[g], bounds[g + 1]) for g in range(G)]

    # one concatenated input: [pts (NC*16) | gx 8 | gy 64 | gz NC*WZ]
    TOT = NC * FEAT + XPER + GY + NC * WZ
    nc = bass.Bass(use_seq_codegen=True)
    inp = nc.declare_dram_parameter("inp", [PPC, TOT], f32, isOutput=False)
    out = nc.declare_dram_parameter("out", [XPER * GY, GZ * GT], f32, isOutput=True)

    with tile.TileContext(nc) as tc, ExitStack() as ctx:
        cpool = ctx.enter_context(tc.tile_pool(name="const", bufs=1))
        ppool = ctx.enter_context(tc.tile_pool(name="accp", bufs=1, space="PSUM"))

        zero_t = cpool.tile([PPC, 640], f16, name="zero_t")
        nc.gpsimd.memset(zero_t[:, :], 0.0)
        dummy_t = cpool.tile([PPC, 1], f16, name="dummy_t")
        nc.scalar.activation(dummy_t[:, :], zero_t[:, 0:1], ACTF.Exp, scale=-0.5)

        inp_t = cpool.tile([PPC, TOT], f32, name="inp_t")
        nc.gpsimd.dma_start(inp_t[:, :], inp[:, :])
        pts3 = inp_t[:, 0 : NC * FEAT].rearrange("p (c f) -> p c f", f=FEAT)
        gx_t = inp_t[:, NC * FEAT : NC * FEAT + XPER]
        gy_t = inp_t[:, NC * FEAT + XPER : NC * FEAT + XPER + GY]
        gz3 = inp_t[:, NC * FEAT + XPER + GY : TOT].rearrange(
            "p (c w) -> p c w", w=WZ
        )

        # PSUM init: contiguous [128, 2048], 4 zero-matmuls
        acc = ppool.tile([128, 4 * 512], f32, name="acc")
        for m in range(4):
            nc.tensor.matmul(
                acc[:, m * 512 : (m + 1) * 512],
                lhsT=zero_t[:, 0:128],
                rhs=zero_t[:, 128:640],
                start=True,
                stop=False,
            )

        # per-point scalars
        inv_t = cpool.tile([PPC, NC, 3], f32, name="inv_t")
        nc.vector.reciprocal(inv_t[:, :, :], pts3[:, :, 11:14])
        iv2_t = cpool.tile([PPC, NC, 3], f32, name="iv2_t")
        nc.vector.tensor_tensor(iv2_t[:, :, :], inv_t[:, :, :], inv_t[:, :, :], AL.mult)
        ivzc_t = cpool.tile([PPC, NC], f32, name="ivzc_t")
        nc.vector.tensor_scalar(ivzc_t[:, :], inv_t[:, :, 2], C0, None, AL.mult)
        m1_t = cpool.tile([PPC, NC], f32, name="m1_t")
        nc.vector.tensor_tensor(m1_t[:, :], inv_t[:, :, 0], inv_t[:, :, 1], AL.mult)
        m2_t = cpool.tile([PPC, NC], f32, name="m2_t")
        nc.vector.tensor_tensor(m2_t[:, :], m1_t[:, :], ivzc_t[:, :], AL.mult)
        xc_t = cpool.tile([PPC, NC, GT], f16, name="xc_t")
        nc.vector.tensor_tensor(
            xc_t[:, :, :],
            pts3[:, :, 0:GT],
            m2_t[:, :].unsqueeze(2).broadcast_to((PPC, NC, GT)),
            AL.mult,
        )
        # iv2 replicated along the grid segments (fp16, 2x_2p copies)
        ivL_t = cpool.tile([PPC, NC, L], f16, name="ivL_t")
        nc.vector.tensor_copy(
            ivL_t[:, :, 0:XPER], iv2_t[:, :, 0:1].broadcast_to((PPC, NC, XPER))
        )
        nc.vector.tensor_copy(
            ivL_t[:, :, XPER:ZO], iv2_t[:, :, 1:2].broadcast_to((PPC, NC, GY))
        )
        nc.vector.tensor_copy(
            ivL_t[:, :, ZO:L], iv2_t[:, :, 2:3].broadcast_to((PPC, NC, WZ))
        )

        d_t = cpool.tile([PPC, NC, L], f16, name="d_t")
        sq_t = cpool.tile([PPC, NC, L], f16, name="sq_t")
        usq_t = cpool.tile([PPC, NC, L], f16, name="usq_t")
        b_t = cpool.tile([PPC, NC, L], f16, name="b_t")
        bxr_t = cpool.tile([PPC, NC, XPER, GY], f16, name="bxr_t")
        bzp_t = cpool.tile([PPC, NC, WZ, 2], f16, name="bzp_t")
        bzr_t = cpool.tile([PPC, NC, WZ, GT], f16, name="bzr_t")
        p_t = cpool.tile([PPC, NC, XPER, GY], f16, name="p_t")
        q_t = cpool.tile([PPC, NC, WZ, GT], f16, name="q_t")
        pf = p_t[:, :, :, :].rearrange("p c a b -> p c (a b)")
        qf = q_t[:, :, :, :].rearrange("p c a b -> p c (a b)")

        def emit_front(g):
            c0, c1 = groups[g]
            n = c1 - c0
            nc.vector.tensor_tensor(
                d_t[:, c0:c1, 0:XPER],
                gx_t.unsqueeze(1).broadcast_to((PPC, n, XPER)),
                pts3[:, c0:c1, 8:9].broadcast_to((PPC, n, XPER)),
                AL.subtract,
            )
            nc.vector.tensor_tensor(
                d_t[:, c0:c1, XPER:ZO],
                gy_t.unsqueeze(1).broadcast_to((PPC, n, GY)),
                pts3[:, c0:c1, 9:10].broadcast_to((PPC, n, GY)),
                AL.subtract,
            )
            nc.vector.tensor_tensor(
                d_t[:, c0:c1, ZO:L],
                gz3[:, c0:c1, :],
                pts3[:, c0:c1, 10:11].broadcast_to((PPC, n, WZ)),
                AL.subtract,
            )
            nc.vector.tensor_tensor(
                sq_t[:, c0:c1, :], d_t[:, c0:c1, :], d_t[:, c0:c1, :], AL.mult
            )
            nc.vector.tensor_tensor(
                usq_t[:, c0:c1, :], sq_t[:, c0:c1, :], ivL_t[:, c0:c1, :], AL.mult
            )
            # ACT: compact exp + bx replicated along y via broadcast-exp
            nc.scalar.activation(
                b_t[:, c0:c1, :], usq_t[:, c0:c1, :], ACTF.Exp, scale=-0.5
            )
            nc.scalar.activation(
                bxr_t[:, c0:c1, :, :],
                usq_t[:, c0:c1, 0:XPER].unsqueeze(3).broadcast_to((PPC, n, XPER, GY)),
                ACTF.Exp,
                scale=-0.5,
            )

        def emit_back(g, last):
            c0, c1 = groups[g]
            n = c1 - c0
            # bz replicated over t: pair-copy then int32 quad-copy (both 2x_2p)
            nc.vector.tensor_copy(
                bzp_t[:, c0:c1, :, :],
                b_t[:, c0:c1, ZO:L].unsqueeze(3).broadcast_to((PPC, n, WZ, 2)),
            )
            bzp32 = bzp_t[:, c0:c1, :, :].rearrange("p c w two -> p c (w two)").bitcast(
                u32
            )
            bzr32 = bzr_t[:, c0:c1, :, :].rearrange("p c w t -> p c (w t)").bitcast(u32)
            nc.vector.tensor_copy(
                bzr32.rearrange("p c (w q) -> p c w q", q=GT // 2),
                bzp32.unsqueeze(3).broadcast_to((PPC, n, WZ, GT // 2)),
            )
            nc.vector.tensor_tensor(
                q_t[:, c0:c1, :, :],
                bzr_t[:, c0:c1, :, :],
                xc_t[:, c0:c1, :].unsqueeze(2).broadcast_to((PPC, n, WZ, GT)),
                AL.mult,
            )
            nc.vector.tensor_tensor(
                p_t[:, c0:c1, :, :],
                bxr_t[:, c0:c1, :, :],
                b_t[:, c0:c1, XPER:ZO].unsqueeze(2).broadcast_to((PPC, n, XPER, GY)),
                AL.mult,
            )
            for c in range(c0, c1):
                z0, wz, s0 = int(z0s[c]), int(wzs[c]), int(s0s[c])
                for m in range(4):
                    nc.tensor.matmul(
                        acc[:, m * 512 + z0 * GT : m * 512 + (z0 + wz) * GT],
                        lhsT=pf[:, c, m * 128 : (m + 1) * 128],
                        rhs=qf[:, c, s0 * GT : (s0 + wz) * GT],
                        start=False,
                        stop=(last and c == c1 - 1),
                    )

        emit_front(0)
        for g in range(1, G):
            emit_front(g)
            emit_back(g - 1, last=False)
        emit_back(G - 1, last=True)

        # evacuate + store, per bank (pipelines with the out DMA)
        o_t = cpool.tile([128, 4 * 512], f32, name="o_t")
        for m in range(4):
            nc.scalar.copy(o_t[:, m * 512 : (m + 1) * 512], acc[:, m * 512 : (m + 1) * 512])
            nc.sync.dma_start(
                out[m * 128 : (m + 1) * 128, :], o_t[:, m * 512 : (m + 1) * 512]
            )

    _split_multi_waits(nc, mybir)
    return nc


def _split_multi_waits(nc, mybir):
    k = 0
    for bb in nc.m.functions[0].blocks:
        new = []
        for inst in bb.instructions:
            si = inst.sync_info
            if si is not None and si.on_wait and len(si.on_wait) > 1:
                for w in si.on_wait[:-1]:
                    wi = mybir.InstEventSemaphore(name=f"wsplit_{k}", ins=[], outs=[])
                    k += 1
                    wi.engine = inst.engine
                    wi.sync_info = mybir.SyncInfo(on_wait=[w], on_update=[])
                    nc.register_instruction(wi)
                    new.append(wi)
                inst.sync_info = mybir.SyncInfo(
                    on_wait=[si.on_wait[-1]], on_update=si.on_update
                )
            new.append(inst)
        bb.instructions[:] = new


def _get_prog(n_chunks, z0s, wzs, s0s, WZ):
    key = (n_chunks, tuple(z0s), tuple(wzs), tuple(s0s), WZ, N_GROUPS, "v4")
    if key not in _prog_cache:
        _prog_cache[key] = _build(n_chunks, z0s, wzs, s0s, WZ)
    return _prog_cache[key]


def _pack_points(x, mu, sigma, n_chunks, z0s, wzs, core):
    n = x.shape[0]
    cap = n_chunks * PPC
    feat = np.zeros((cap, FEAT), np.float32)
    feat[:, 11:14] = 1.0
    for c in range(n_chunks):
        feat[c * PPC : (c + 1) * PPC, 8] = core * XPER + XPER / 2.0
        feat[c * PPC : (c + 1) * PPC, 9] = GY / 2.0
        feat[c * PPC : (c + 1) * PPC, 10] = z0s[c] + wzs[c] / 2.0
    feat[:n, 0:8] = x
    feat[:n, 8:11] = mu
    feat[:n, 11:14] = sigma
    return (
        feat.reshape(n_chunks, PPC, FEAT).transpose(1, 0, 2).reshape(PPC, n_chunks * FEAT)
    )


def _prepare(x, mu, sigma):
    n = x.shape[0]
    C = SIGMA_CUT
    sel = []
    for c in range(N_CORES):
        lo, hi = c * XPER, c * XPER + XPER - 1
        d = np.maximum.reduce([lo - mu[:, 0], mu[:, 0] - hi, np.zeros(n, np.float32)])
        idx = np.nonzero(d <= C * sigma[:, 0])[0]
        idx = idx[np.argsort(mu[idx, 2], kind="stable")]
        sel.append(idx)
    n_chunks = max(1, int(np.ceil(max(len(s) for s in sel) / PPC)))

    z0s, z1s = [], []
    for c in range(n_chunks):
        zlo, zhi = GZ, 0
        for k in range(N_CORES):
            idx = sel[k][c * PPC : (c + 1) * PPC]
            if len(idx):
                zlo = min(zlo, np.min(mu[idx, 2] - C * sigma[idx, 2]))
                zhi = max(zhi, np.max(mu[idx, 2] + C * sigma[idx, 2]))
        z0 = max(0, int(np.floor(zlo)))
        z1 = min(GZ, int(np.ceil(zhi)))
        if z1 <= z0:
            z0, z1 = 0, 1
        z0s.append(z0)
        z1s.append(z1)
    wzs = [z1s[c] - z0s[c] for c in range(n_chunks)]
    WZ = max(wzs)
    g0s = [z0s[c] if z0s[c] + WZ <= GZ else GZ - WZ for c in range(n_chunks)]
    s0s = [z0s[c] - g0s[c] for c in range(n_chunks)]

    gy = np.arange(GY, dtype=np.float32)
    in_maps = []
    for k in range(N_CORES):
        gz = np.concatenate(
            [np.arange(g0s[c], g0s[c] + WZ, dtype=np.float32) for c in range(n_chunks)]
        )
        idx = sel[k]
        row = np.concatenate(
            [
                np.zeros(n_chunks * FEAT, np.float32),  # pts placeholder
                np.arange(k * XPER, (k + 1) * XPER, dtype=np.float32),
                gy,
                gz,
            ]
        )
        inp = np.tile(row, (PPC, 1))
        inp[:, 0 : n_chunks * FEAT] = _pack_points(
            x[idx], mu[idx], sigma[idx], n_chunks, z0s, wzs, k
        )
        in_maps.append({"inp": inp})
    return in_maps, n_chunks, z0s, wzs, s0s, WZ


def _assemble(results):
    o = np.stack([results[c]["out"] for c in range(N_CORES)])  # [8, 512, 512]
    o = o.reshape(N_CORES, XPER, GY, GZ, GT)
    return np.ascontiguousarray(o.reshape(GX, GY, GZ, GT))


def run(x, mu, sigma, trace=False, **spmd_kwargs):
    from concourse.bass_utils import run_bass_kernel_spmd

    x = np.asarray(x, np.float32)
    mu = np.asarray(mu, np.float32)
    sigma = np.asarray(sigma, np.float32)
    in_maps, n_chunks, z0s, wzs, s0s, WZ = _prepare(x, mu, sigma)
    nc = _get_prog(n_chunks, z0s, wzs, s0s, WZ)
    res = run_bass_kernel_spmd(
        nc, in_maps, list(range(N_CORES)), trace=trace, **spmd_kwargs
    )
    return _assemble(res.results), res


def kernel(x, mu, sigma):
    out, _ = run(x, mu, sigma)
    return out


# revision 9
# speedup vs baseline: 1.3206x; 1.0017x over previous
"""Trainium2 Bass kernel for nn_Kernel3D (Gaussian splat onto a 64x64x64x8 grid).

Math:  out[x,y,z,t] = sum_n bx[n,x] * by[n,y] * bz[n,z] * x[n,t]
where b?[n,g] = exp(-0.5*((g-mu)/s)^2) / sqrt(2*pi*s^2).

v5: x-slab sharding (8 planes/core) + z-sorted point chunks of 128 with a
uniform z-window width WZ (chunk z-offsets shared across cores so the SPMD
program is identical).  Per chunk the accumulated matmul is
    out[(x y), (z t)] += P[n, (x y)]^T @ Q[n, zwin t]
PSUM is organized as 8 banks = (xy-block, z-half): because chunks are
z-sorted, the z<32 half of the output is final mid-stream and its
evacuation + store DMA overlap the remaining matmuls (different banks, so
no PE/ACT bank collisions).

Engine split (all build work batched into large per-GROUP instructions):
  DVE   d = g - mu; sqd = d*d (fp16 2x); usq = sqd * iv2-replica (2x);
        bz-replica over t via int32-bitcast pair/quad copies (2x_2p);
        P and Q outer-product TTs at 2x; half the PSUM evacuation
  ACT   one batched exp per group; fused broadcast-exp building bx
        replicated along y (so P's TT runs at 2x); half the evacuation
  PE    8 zero-matmuls init PSUM; per chunk 4 x (1..2) windowed fp16 MMs
  GPSIMD tiny per-point scalar ops, memset, input DMA, half the out DMA

Host side: selection (slab dist <= C*sigma_x), z-sort, packing, window
metadata.  No host math on values.
"""

import os
import sys

import numpy as np

for _p in ("/opt/trn_rl_repo", "/root/.axon_site/_ro/trn_rl_repo"):
    if os.path.isdir(_p) and _p not in sys.path:
        sys.path.insert(0, _p)

N_CORES = 8
GX, GY, GZ, GT = 64, 64, 64, 8
XPER = GX // N_CORES
PPC = 128
FEAT = 16  # x[8], mu[3], sigma[3], pad[2]
ZH = GZ // 2  # z-half boundary

SIGMA_CUT = 3.0
N_GROUPS = 3

_prog_cache = {}


def _build(n_chunks, z0s, z1s, g0s, WZ):
    import concourse.bass as bass
    import concourse.tile as tile
    from concourse import mybir
    from contextlib import ExitStack

    f32 = mybir.dt.float32
    f16 = mybir.dt.float16
    u32 = mybir.dt.uint32
    AL = mybir.AluOpType
    ACTF = mybir.ActivationFunctionType
    C0 = float((2.0 * np.pi) ** -1.5)
    NC = n_chunks
    L = XPER + GY + WZ
    ZO = XPER + GY

    G = min(N_GROUPS, NC)
    bounds = [round(g * NC / G) for g in range(G + 1)]
    groups = [(bounds[g], bounds[g + 1]) for g in range(G)]

    # per-chunk (half, zlo, zhi) matmul parts; last chunk touching each half
    parts = []
    for c in range(NC):
        pr = []
        for h in (0, 1):
            zlo, zhi = max(z0s[c], ZH * h), min(z1s[c], ZH * (h + 1))
            if zhi > zlo:
                pr.append((h, zlo, zhi))
        parts.append(pr)
    last_touch = {h: max(c for c in range(NC) if any(p[0] == h for p in parts[c]))
                  for h in (0, 1)}

    TOT = NC * FEAT + XPER + GY + NC * WZ
    NPTS = NC * FEAT
    nc = bass.Bass(use_seq_codegen=True)
    inp = nc.declare_dram_parameter("inp", [PPC, TOT], f32, isOutput=False)
    out = nc.declare_dram_parameter("out", [XPER * GY, GZ * GT], f32, isOutput=True)

    with tile.TileContext(nc) as tc, ExitStack() as ctx:
        cpool = ctx.enter_context(tc.tile_pool(name="const", bufs=1))
        ppool = ctx.enter_context(tc.tile_pool(name="accp", bufs=1, space="PSUM"))

        zero_t = cpool.tile([PPC, 640], f16, name="zero_t")
        nc.gpsimd.memset(zero_t[:, :], 0.0)
        dummy_t = cpool.tile([PPC, 1], f16, name="dummy_t")
        nc.scalar.activation(dummy_t[:, :], zero_t[:, 0:1], ACTF.Exp, scale=-0.5)

        inp_t = cpool.tile([PPC, TOT], f32, name="inp_t")
        nc.gpsimd.dma_start(inp_t[:, 0:NPTS], inp[:, 0:NPTS])
        nc.sync.dma_start(inp_t[:, NPTS:TOT], inp[:, NPTS:TOT])
        pts3 = inp_t[:, 0:NPTS].rearrange("p (c f) -> p c f", f=FEAT)
        gx_t = inp_t[:, NPTS : NPTS + XPER]
        gy_t = inp_t[:, NPTS + XPER : NPTS + XPER + GY]
        gz3 = inp_t[:, NPTS + XPER + GY : TOT].rearrange("p (c w) -> p c w", w=WZ)

        # PSUM: 8 banks; bank (m, h) at columns (2m+h)*512, 256 cols used
        acc = ppool.tile([128, 8 * 512], f32, name="acc")
        for b in range(8):
            nc.tensor.matmul(
                acc[:, b * 512 : b * 512 + ZH * GT],
                lhsT=zero_t[:, 0:128],
                rhs=zero_t[:, 128 : 128 + ZH * GT],
                start=True,
                stop=False,
            )

        # per-point scalars: recip on DVE, tiny follow-ups on GPSIMD
        inv_t = cpool.tile([PPC, NC, 3], f32, name="inv_t")
        iv2_t = cpool.tile([PPC, NC, 3], f32, name="iv2_t")
        ivzc_t = cpool.tile([PPC, NC], f32, name="ivzc_t")
        m1_t = cpool.tile([PPC, NC], f32, name="m1_t")
        m2_t = cpool.tile([PPC, NC], f32, name="m2_t")
        xc_t = cpool.tile([PPC, NC, GT], f16, name="xc_t")
        ivL_t = cpool.tile([PPC, NC, L], f16, name="ivL_t")

        def emit_pre():
            nc.vector.reciprocal(inv_t[:, :, :], pts3[:, :, 11:14])
            nc.gpsimd.tensor_tensor(
                iv2_t[:, :, :], inv_t[:, :, :], inv_t[:, :, :], AL.mult
            )
            nc.gpsimd.tensor_scalar(ivzc_t[:, :], inv_t[:, :, 2], C0, None, AL.mult)
            nc.gpsimd.tensor_tensor(m1_t[:, :], inv_t[:, :, 0], inv_t[:, :, 1], AL.mult)
            nc.gpsimd.tensor_tensor(m2_t[:, :], m1_t[:, :], ivzc_t[:, :], AL.mult)
            nc.vector.tensor_tensor(
                xc_t[:, :, :],
                pts3[:, :, 0:GT],
                m2_t[:, :].unsqueeze(2).broadcast_to((PPC, NC, GT)),
                AL.mult,
            )
            nc.vector.tensor_copy(
                ivL_t[:, :, 0:XPER], iv2_t[:, :, 0:1].broadcast_to((PPC, NC, XPER))
            )
            nc.vector.tensor_copy(
                ivL_t[:, :, XPER:ZO], iv2_t[:, :, 1:2].broadcast_to((PPC, NC, GY))
            )
            nc.vector.tensor_copy(
                ivL_t[:, :, ZO:L], iv2_t[:, :, 2:3].broadcast_to((PPC, NC, WZ))
            )

        d_t = cpool.tile([PPC, NC, L], f16, name="d_t")
        sq_t = cpool.tile([PPC, NC, L], f16, name="sq_t")
        usq_t = cpool.tile([PPC, NC, L], f16, name="usq_t")
        b_t = cpool.tile([PPC, NC, L], f16, name="b_t")
        bxr_t = cpool.tile([PPC, NC, XPER, GY], f16, name="bxr_t")
        bzp_t = cpool.tile([PPC, NC, WZ, 2], f16, name="bzp_t")
        bzr_t = cpool.tile([PPC, NC, WZ, GT], f16, name="bzr_t")
        p_t = cpool.tile([PPC, NC, XPER, GY], f16, name="p_t")
        q_t = cpool.tile([PPC, NC, WZ, GT], f16, name="q_t")
        pf = p_t[:, :, :, :].rearrange("p c a b -> p c (a b)")
        qf = q_t[:, :, :, :].rearrange("p c a b -> p c (a b)")
        o_t = cpool.tile([128, 8 * ZH * GT], f32, name="o_t")

        def emit_d(g):
            c0, c1 = groups[g]
            n = c1 - c0
            nc.vector.tensor_tensor(
                d_t[:, c0:c1, 0:XPER],
                gx_t.unsqueeze(1).broadcast_to((PPC, n, XPER)),
                pts3[:, c0:c1, 8:9].broadcast_to((PPC, n, XPER)),
                AL.subtract,
            )
            nc.vector.tensor_tensor(
                d_t[:, c0:c1, XPER:ZO],
                gy_t.unsqueeze(1).broadcast_to((PPC, n, GY)),
                pts3[:, c0:c1, 9:10].broadcast_to((PPC, n, GY)),
                AL.subtract,
            )
            nc.vector.tensor_tensor(
                d_t[:, c0:c1, ZO:L],
                gz3[:, c0:c1, :],
                pts3[:, c0:c1, 10:11].broadcast_to((PPC, n, WZ)),
                AL.subtract,
            )

        def emit_front(g):
            c0, c1 = groups[g]
            n = c1 - c0
            nc.vector.tensor_tensor(
                sq_t[:, c0:c1, :], d_t[:, c0:c1, :], d_t[:, c0:c1, :], AL.mult
            )
            nc.vector.tensor_tensor(
                usq_t[:, c0:c1, :], sq_t[:, c0:c1, :], ivL_t[:, c0:c1, :], AL.mult
            )
            nc.scalar.activation(
                b_t[:, c0:c1, :], usq_t[:, c0:c1, :], ACTF.Exp, scale=-0.5
            )
            nc.scalar.activation(
                bxr_t[:, c0:c1, :, :],
                usq_t[:, c0:c1, 0:XPER].unsqueeze(3).broadcast_to((PPC, n, XPER, GY)),
                ACTF.Exp,
                scale=-0.5,
            )

        def emit_evac(h, engines):
            # evacuate + store the 4 (m, h) banks (256 used cols each)
            W = ZH * GT
            for m in range(4):
                b = 2 * m + h
                eng = engines[m % 2]
                if eng == "act":
                    nc.scalar.copy(
                        o_t[:, b * W : (b + 1) * W], acc[:, b * 512 : b * 512 + W]
                    )
                else:
                    nc.vector.tensor_copy(
                        o_t[:, b * W : (b + 1) * W], acc[:, b * 512 : b * 512 + W]
                    )
                dq = nc.sync if m % 2 == 0 else nc.gpsimd
                dq.dma_start(
                    out[m * 128 : (m + 1) * 128, h * W : (h + 1) * W],
                    o_t[:, b * W : (b + 1) * W],
                )

        def emit_back(g):
            c0, c1 = groups[g]
            n = c1 - c0
            nc.vector.tensor_copy(
                bzp_t[:, c0:c1, :, :],
                b_t[:, c0:c1, ZO:L].unsqueeze(3).broadcast_to((PPC, n, WZ, 2)),
            )
            bzp32 = bzp_t[:, c0:c1, :, :].rearrange("p c w two -> p c (w two)").bitcast(
                u32
            )
            bzr32 = bzr_t[:, c0:c1, :, :].rearrange("p c w t -> p c (w t)").bitcast(u32)
            nc.vector.tensor_copy(
                bzr32.rearrange("p c (w q) -> p c w q", q=GT // 2),
                bzp32.unsqueeze(3).broadcast_to((PPC, n, WZ, GT // 2)),
            )
            nc.vector.tensor_tensor(
                q_t[:, c0:c1, :, :],
                bzr_t[:, c0:c1, :, :],
                xc_t[:, c0:c1, :].unsqueeze(2).broadcast_to((PPC, n, WZ, GT)),
                AL.mult,
            )
            nc.vector.tensor_tensor(
                p_t[:, c0:c1, :, :],
                bxr_t[:, c0:c1, :, :],
                b_t[:, c0:c1, XPER:ZO].unsqueeze(2).broadcast_to((PPC, n, XPER, GY)),
                AL.mult,
            )
            for c in range(c0, c1):
                for (h, zlo, zhi) in parts[c]:
                    s = zlo - g0s[c]
                    w = zhi - zlo
                    stop = c == last_touch[h]
                    for m in range(4):
                        b = 2 * m + h
                        nc.tensor.matmul(
                            acc[
                                :,
                                b * 512 + (zlo - ZH * h) * GT : b * 512
                                + (zhi - ZH * h) * GT,
                            ],
                            lhsT=pf[:, c, m * 128 : (m + 1) * 128],
                            rhs=qf[:, c, s * GT : (s + w) * GT],
                            start=False,
                            stop=stop and m == 3,
                        )
                if c == last_touch[0]:
                    emit_evac(0, ("act", "act"))

        emit_pre()
        emit_d(0)
        emit_front(0)
        for g in range(1, G):
            emit_d(g)
            emit_front(g)
            emit_back(g - 1)
        emit_back(G - 1)
        emit_evac(1, ("act", "dve"))

    _split_multi_waits(nc, mybir)
    return nc


def _split_multi_waits(nc, mybir):
    k = 0
    for bb in nc.m.functions[0].blocks:
        new = []
        for inst in bb.instructions:
            si = inst.sync_info
            if si is not None and si.on_wait and len(si.on_wait) > 1:
                for w in si.on_wait[:-1]:
                    wi = mybir.InstEventSemaphore(name=f"wsplit_{k}", ins=[], outs=[])
                    k += 1
                    wi.engine = inst.engine
                    wi.sync_info = mybir.SyncInfo(on_wait=[w], on_update=[])
                    nc.register_instruction(wi)
                    new.append(wi)
                inst.sync_info = mybir.SyncInfo(
                    on_wait=[si.on_wait[-1]], on_update=si.on_update
                )
            new.append(inst)
        bb.instructions[:] = new


def _get_prog(n_chunks, z0s, z1s, g0s, WZ):
    key = (n_chunks, tuple(z0s), tuple(z1s), tuple(g0s), WZ, N_GROUPS, "v5")
    if key not in _prog_cache:
        _prog_cache[key] = _build(n_chunks, z0s, z1s, g0s, WZ)
    return _prog_cache[key]


def _pack_points(x, mu, sigma, n_chunks, z0s, wzs, core):
    n = x.shape[0]
    cap = n_chunks * PPC
    feat = np.zeros((cap, FEAT), np.float32)
    feat[:, 11:14] = 1.0
    for c in range(n_chunks):
        feat[c * PPC : (c + 1) * PPC, 8] = core * XPER + XPER / 2.0
        feat[c * PPC : (c + 1) * PPC, 9] = GY / 2.0
        feat[c * PPC : (c + 1) * PPC, 10] = z0s[c] + wzs[c] / 2.0
    feat[:n, 0:8] = x
    feat[:n, 8:11] = mu
    feat[:n, 11:14] = sigma
    return (
        feat.reshape(n_chunks, PPC, FEAT).transpose(1, 0, 2).reshape(PPC, n_chunks * FEAT)
    )


def _prepare(x, mu, sigma):
    n = x.shape[0]
    C = SIGMA_CUT
    sel = []
    for c in range(N_CORES):
        lo, hi = c * XPER, c * XPER + XPER - 1
        d = np.maximum.reduce([lo - mu[:, 0], mu[:, 0] - hi, np.zeros(n, np.float32)])
        idx = np.nonzero(d <= C * sigma[:, 0])[0]
        idx = idx[np.argsort(mu[idx, 2], kind="stable")]
        sel.append(idx)
    n_chunks = max(1, int(np.ceil(max(len(s) for s in sel) / PPC)))

    z0s, z1s = [], []
    for c in range(n_chunks):
        zlo, zhi = GZ, 0
        for k in range(N_CORES):
            idx = sel[k][c * PPC : (c + 1) * PPC]
            if len(idx):
                zlo = min(zlo, np.min(mu[idx, 2] - C * sigma[idx, 2]))
                zhi = max(zhi, np.max(mu[idx, 2] + C * sigma[idx, 2]))
        z0 = max(0, int(np.floor(zlo)))
        z1 = min(GZ, int(np.ceil(zhi)))
        if z1 <= z0:
            z0, z1 = 0, 1
        z0s.append(z0)
        z1s.append(z1)
    wzs = [z1s[c] - z0s[c] for c in range(n_chunks)]
    WZ = max(wzs)
    g0s = [z0s[c] if z0s[c] + WZ <= GZ else GZ - WZ for c in range(n_chunks)]

    gy = np.arange(GY, dtype=np.float32)
    in_maps = []
    for k in range(N_CORES):
        gz = np.concatenate(
            [np.arange(g0s[c], g0s[c] + WZ, dtype=np.float32) for c in range(n_chunks)]
        )
        idx = sel[k]
        row = np.concatenate(
            [
                np.zeros(n_chunks * FEAT, np.float32),
                np.arange(k * XPER, (k + 1) * XPER, dtype=np.float32),
                gy,
                gz,
            ]
        )
        inp = np.tile(row, (PPC, 1))
        inp[:, 0 : n_chunks * FEAT] = _pack_points(
            x[idx], mu[idx], sigma[idx], n_chunks, z0s, wzs, k
        )
        in_maps.append({"inp": inp})
    return in_maps, n_chunks, z0s, z1s, g0s, WZ


def _assemble(results):
    o = np.stack([results[c]["out"] for c in range(N_CORES)])  # [8, 512, 512]
    o = o.reshape(N_CORES, XPER, GY, GZ, GT)
    return np.ascontiguousarray(o.reshape(GX, GY, GZ, GT))


def run(x, mu, sigma, trace=False, **spmd_kwargs):
    from concourse.bass_utils import run_bass_kernel_spmd

    x = np.asarray(x, np.float32)
    mu = np.asarray(mu, np.float32)
    sigma = np.asarray(sigma, np.float32)
    in_maps, n_chunks, z0s, z1s, g0s, WZ = _prepare(x, mu, sigma)
    nc = _get_prog(n_chunks, z0s, z1s, g0s, WZ)
    res = run_bass_kernel_spmd(
        nc, in_maps, list(range(N_CORES)), trace=trace, **spmd_kwargs
    )
    return _assemble(res.results), res


def kernel(x, mu, sigma):
    out, _ = run(x, mu, sigma)
    return out
